# revision 4
# baseline (speedup 1.0000x reference)
"""Bahdanau-style additive attention kernel for Trainium2 (8 NeuronCores).

Problem: enc_hidden [32, 2048, 512], dec_hidden [32, 512], W/W1/v weights.
  enc_proj = enc @ Ww + Wb            [B, S, U]
  dec_proj = dec @ W1w + W1b          [B, 1, U]
  unnorm   = tanh(enc_proj + dec_proj) @ vw + vb   [B, S]
  attn     = softmax(unnorm, axis=S)
  c_vec    = sum_s attn * enc_hidden  [B, H]

Sharding: pure data parallel over batch — 4 batches per core, weights
replicated. All heavy compute in fp16 on the tensor engine; the enc
transpose (H must sit on SBUF partitions for the contraction) is done
with identity-matmul transposes on the PE.
"""

import numpy as np

B, S, H, U = 32, 2048, 512, 512
NCORES = 8
B_LOC = B // NCORES          # 4 batches per core
NS = S // 128                # 16 s-tiles of 128
NK = H // 128                # 4 contraction chunks
NU = U // 128                # 4 u chunks
SC = 512                     # moving free-dim per matmul
NSC = S // SC                # 4 s-chunks

_CACHE = {}


def _build():
    import concourse.bacc as bacc
    import concourse.tile as tile
    from concourse import mybir
    from concourse.masks import make_identity
    from contextlib import ExitStack

    F32 = mybir.dt.float32
    F16 = mybir.dt.float16
    AF = mybir.ActivationFunctionType

    nc = bacc.Bacc("TRN2", target_bir_lowering=False, debug=False,
                   num_devices=NCORES)

    enc_d = nc.dram_tensor("enc", [B_LOC, S, H], F32, kind="ExternalInput")
    dec_d = nc.dram_tensor("dec", [B_LOC, H], F32, kind="ExternalInput")
    ww_d = nc.dram_tensor("ww", [H, U], F32, kind="ExternalInput")
    wb_d = nc.dram_tensor("wb", [U], F32, kind="ExternalInput")
    w1_d = nc.dram_tensor("w1", [H, U], F32, kind="ExternalInput")
    w1b_d = nc.dram_tensor("w1b", [U], F32, kind="ExternalInput")
    vw_d = nc.dram_tensor("vw", [U, 1], F32, kind="ExternalInput")
    vb_d = nc.dram_tensor("vb", [1], F32, kind="ExternalInput")
    cv_d = nc.dram_tensor("cvec", [B_LOC, H], F32, kind="ExternalOutput")
    un_d = nc.dram_tensor("unnorm", [B_LOC, S], F32, kind="ExternalOutput")
    at_d = nc.dram_tensor("attn", [B_LOC, S], F32, kind="ExternalOutput")

    es = ExitStack()
    with tile.TileContext(nc) as tc:
        wp = es.enter_context(tc.tile_pool(name="wp", bufs=1))
        npool = es.enter_context(tc.tile_pool(name="npool", bufs=2))
        tpool = es.enter_context(tc.tile_pool(name="tpool", bufs=2))
        thp = es.enter_context(tc.tile_pool(name="thp", bufs=3))
        sp = es.enter_context(tc.tile_pool(name="sp", bufs=2))
        ptr = es.enter_context(tc.tile_pool(name="ptr", bufs=2, space="PSUM"))
        pmm = es.enter_context(tc.tile_pool(name="pmm", bufs=3, space="PSUM"))
        pvw = es.enter_context(tc.tile_pool(name="pvw", bufs=1, space="PSUM"))
        pcv = es.enter_context(tc.tile_pool(name="pcv", bufs=1, space="PSUM"))
        psm = es.enter_context(tc.tile_pool(name="psm", bufs=1, space="PSUM"))

        # ---- weights / setup ----
        ident16 = wp.tile([128, 128], F16)
        make_identity(nc, ident16[:])
        i4 = wp.tile([4, 4], F32)
        make_identity(nc, i4[:])

        ww32 = wp.tile([128, NK, U], F32)
        nc.sync.dma_start(out=ww32[:], in_=ww_d.ap().rearrange("(kt p) u -> p kt u", p=128))
        ww16 = wp.tile([128, NK, U], F16)
        nc.vector.tensor_copy(ww16[:], ww32[:])
        w132 = wp.tile([128, NK, U], F32)
        nc.sync.dma_start(out=w132[:], in_=w1_d.ap().rearrange("(kt p) u -> p kt u", p=128))
        vw32 = wp.tile([128, NU], F32)
        nc.sync.dma_start(out=vw32[:], in_=vw_d.ap().rearrange("(uc p) 1 -> p uc", p=128))
        vw16 = wp.tile([128, NU], F16)
        nc.vector.tensor_copy(vw16[:], vw32[:])
        wbsum = wp.tile([128, NU], F32)
        wb2 = wp.tile([128, NU], F32)
        nc.sync.dma_start(out=wbsum[:], in_=wb_d.ap().rearrange("(uc p) -> p uc", p=128))
        nc.sync.dma_start(out=wb2[:], in_=w1b_d.ap().rearrange("(uc p) -> p uc", p=128))
        nc.vector.tensor_add(wbsum[:], wbsum[:], wb2[:])
        vb_t = wp.tile([1, 1], F32)
        nc.sync.dma_start(out=vb_t[:], in_=vb_d[:])
        ones_col = wp.tile([128, 1], F32)
        nc.vector.memset(ones_col[:], 1.0)
        ones_row = wp.tile([1, 128], F32)
        nc.vector.memset(ones_row[:], 1.0)

        # dec_projT -> bias_col[u%128, uc, b] = (dec @ W1w)[b, u] + Wb[u] + W1b[u]
        dec_nat = wp.tile([B_LOC, H], F32)
        nc.sync.dma_start(out=dec_nat[:], in_=dec_d[:])
        decT = wp.tile([128, NK, B_LOC], F32)
        for kt in range(NK):
            pt = psm.tile([128, B_LOC], F32, tag="psm")
            nc.tensor.matmul(pt[:], dec_nat[:, kt * 128:(kt + 1) * 128], i4[:],
                             start=True, stop=True)
            nc.scalar.activation(out=decT[:, kt, :], in_=pt[:], func=AF.Copy)
        bias_col = wp.tile([128, NU, B_LOC], F32)
        for uc in range(NU):
            pb = psm.tile([128, B_LOC], F32, tag="psm")
            for kt in range(NK):
                nc.tensor.matmul(pb[:], w132[:, kt, uc * 128:(uc + 1) * 128],
                                 decT[:, kt, :], start=(kt == 0), stop=(kt == NK - 1))
            nc.scalar.activation(out=bias_col[:, uc, :], in_=pb[:],
                                 func=AF.Identity, bias=wbsum[:, uc:uc + 1], scale=1.0)

        # ---- per-batch pipeline ----
        for b in range(B_LOC):
            nat16 = npool.tile([128, NS, H], F16, tag="nat")
            nc.gpsimd.dma_start(out=nat16[:],
                                in_=enc_d[b].rearrange("(si p) h -> p si h", p=128))

            # transpose: encT[h%128, hc, s] via identity matmuls
            encT = tpool.tile([128, NK, S], F16, tag="encT")
            for si in range(NS):
                pt = ptr.tile([128, 512], F32, tag="ptr")
                for hc in range(NK):
                    nc.tensor.matmul(pt[:, hc * 128:(hc + 1) * 128],
                                     nat16[:, si, hc * 128:(hc + 1) * 128],
                                     ident16[:], start=True, stop=True)
                dst = encT[:, :, si * 128:(si + 1) * 128]
                src_ap = pt[:].rearrange("p (hc q) -> p hc q", hc=NK)
                if si % 2 == 0:
                    nc.vector.tensor_copy(dst, src_ap)
                else:
                    nc.scalar.activation(out=dst, in_=src_ap, func=AF.Copy)

            un_row = sp.tile([1, S], F32, tag="unrow")
            for sc in range(NSC):
                th = thp.tile([128, NU, SC], F16, tag="tanh")
                for uc in range(NU):
                    pm = pmm.tile([128, SC], F32, tag="pmm")
                    for kt in range(NK):
                        nc.tensor.matmul(pm[:], ww16[:, kt, uc * 128:(uc + 1) * 128],
                                         encT[:, kt, sc * SC:(sc + 1) * SC],
                                         start=(kt == 0), stop=(kt == NK - 1))
                    nc.scalar.activation(out=th[:, uc, :], in_=pm[:], func=AF.Tanh,
                                         bias=bias_col[:, uc, b:b + 1], scale=1.0)
                pv = pvw.tile([1, SC], F32, tag="pvw")
                for uc in range(NU):
                    nc.tensor.matmul(pv[:], vw16[:, uc:uc + 1], th[:, uc, :],
                                     start=(uc == 0), stop=(uc == NU - 1))
                nc.scalar.activation(out=un_row[:, sc * SC:(sc + 1) * SC], in_=pv[:],
                                     func=AF.Identity, bias=vb_t[:], scale=1.0)
            nc.sync.dma_start(out=un_d[b:b + 1, :], in_=un_row[:])

            # softmax over all 2048 (no max-sub: |unnorm| <= sum|vw| bounded)
            u_col = sp.tile([128, NS], F32, tag="ucol")
            nc.sync.dma_start(out=u_col[:],
                              in_=un_d[b:b + 1, :].rearrange("1 (si p) -> p si", p=128))
            e_col = sp.tile([128, NS], F32, tag="ecol")
            epart = sp.tile([128, 1], F32, tag="epart")
            nc.scalar.activation(out=e_col[:], in_=u_col[:], func=AF.Exp,
                                 accum_out=epart[:])
            pz = psm.tile([1, 1], F32, tag="psm")
            nc.tensor.matmul(pz[:], ones_col[:], epart[:], start=True, stop=True)
            z_sb = sp.tile([1, 1], F32, tag="zsb")
            nc.vector.tensor_copy(z_sb[:], pz[:])
            rz = sp.tile([1, 1], F32, tag="rz")
            nc.vector.reciprocal(rz[:], z_sb[:])
            prz = psm.tile([128, 1], F32, tag="psm")
            nc.tensor.matmul(prz[:], ones_row[:], rz[:], start=True, stop=True)
            rz_col = sp.tile([128, 1], F32, tag="rzcol")
            nc.vector.tensor_copy(rz_col[:], prz[:])
            at_col = sp.tile([128, NS], F32, tag="atcol")
            nc.vector.tensor_scalar_mul(at_col[:], e_col[:], rz_col[:])
            at16 = sp.tile([128, NS], F16, tag="at16")
            nc.vector.tensor_copy(at16[:], at_col[:])
            nc.sync.dma_start(out=at_d[b:b + 1, :].rearrange("1 (si p) -> p si", p=128),
                              in_=at_col[:])

            # c_vec = sum_s attn[s] * enc[s, :]
            pc = pcv.tile([1, H], F32, tag="pcv")
            for si in range(NS):
                nc.tensor.matmul(pc[:], at16[:, si:si + 1], nat16[:, si, :],
                                 start=(si == 0), stop=(si == NS - 1))
            cv_row = sp.tile([1, H], F32, tag="cvrow")
            nc.scalar.activation(out=cv_row[:], in_=pc[:], func=AF.Copy)
            nc.sync.dma_start(out=cv_d[b:b + 1, :], in_=cv_row[:])
        es.close()

    nc.compile()
    return nc


def get_nc():
    if "nc" not in _CACHE:
        _CACHE["nc"] = _build()
    return _CACHE["nc"]


def make_in_maps(enc_hidden, dec_hidden, Ww, Wb, W1w, W1b, vw, vb):
    enc = np.ascontiguousarray(np.asarray(enc_hidden, dtype=np.float32))
    dec = np.ascontiguousarray(np.asarray(dec_hidden, dtype=np.float32))
    ww = np.ascontiguousarray(np.asarray(Ww, dtype=np.float32))
    wb = np.ascontiguousarray(np.asarray(Wb, dtype=np.float32))
    w1 = np.ascontiguousarray(np.asarray(W1w, dtype=np.float32))
    w1b = np.ascontiguousarray(np.asarray(W1b, dtype=np.float32))
    vww = np.ascontiguousarray(np.asarray(vw, dtype=np.float32))
    vbb = np.ascontiguousarray(np.asarray(vb, dtype=np.float32))
    in_maps = []
    for c in range(NCORES):
        sl = slice(c * B_LOC, (c + 1) * B_LOC)
        in_maps.append({
            "enc": np.ascontiguousarray(enc[sl]),
            "dec": np.ascontiguousarray(dec[sl]),
            "ww": ww, "wb": wb, "w1": w1, "w1b": w1b,
            "vw": vww, "vb": vbb,
        })
    return in_maps


def kernel(enc_hidden, dec_hidden, Ww, Wb, W1w, W1b, vw, vb, _trace=False):
    from concourse.bass_utils import run_bass_kernel_spmd

    nc = get_nc()
    in_maps = make_in_maps(enc_hidden, dec_hidden, Ww, Wb, W1w, W1b, vw, vb)
    res = run_bass_kernel_spmd(nc, in_maps, list(range(NCORES)), trace=_trace)
    c_vec = np.concatenate([res.results[c]["cvec"] for c in range(NCORES)], axis=0)
    unnorm = np.concatenate([res.results[c]["unnorm"] for c in range(NCORES)], axis=0)
    attn = np.concatenate([res.results[c]["attn"] for c in range(NCORES)], axis=0)
    out = (c_vec.astype(np.float32), unnorm.astype(np.float32), attn.astype(np.float32))
    if _trace:
        return out, res
    return out


# revision 5
# speedup vs baseline: 1.0325x; 1.0325x over previous
"""Bahdanau-style additive attention kernel for Trainium2 (8 NeuronCores).

Problem: enc_hidden [32, 2048, 512], dec_hidden [32, 512], W/W1/v weights.
  enc_proj = enc @ Ww + Wb            [B, S, U]
  dec_proj = dec @ W1w + W1b          [B, 1, U]
  unnorm   = tanh(enc_proj + dec_proj) @ vw + vb   [B, S]
  attn     = softmax(unnorm, axis=S)
  c_vec    = sum_s attn * enc_hidden  [B, H]

Sharding: pure data parallel over batch — 4 batches per core, weights
replicated. Heavy compute in fp16 on the tensor engine; enc is
transposed on-chip with PE transpose-mode ops (H must sit on SBUF
partitions for the contraction). Softmax is computed in a column layout
so all 128 lanes participate; exp needs no max-subtraction because
|unnorm| <= sum|vw| is bounded by construction (tanh in [-1,1]).
"""

import numpy as np

B, S, H, U = 32, 2048, 512, 512
NCORES = 8
B_LOC = B // NCORES          # 4 batches per core
NS = S // 128                # 16 s-tiles of 128
NK = H // 128                # 4 contraction chunks
NU = U // 128                # 4 u chunks
SC = 512                     # moving free-dim per matmul
NSC = S // SC                # 4 s-chunks

_CACHE = {}


def _build():
    import concourse.bacc as bacc
    import concourse.tile as tile
    from concourse import mybir
    from concourse.masks import make_identity
    from contextlib import ExitStack

    F32 = mybir.dt.float32
    F16 = mybir.dt.float16
    AF = mybir.ActivationFunctionType

    nc = bacc.Bacc("TRN2", target_bir_lowering=False, debug=False,
                   num_devices=NCORES)

    enc_d = nc.dram_tensor("enc", [B_LOC, S, H], F32, kind="ExternalInput")
    dec_d = nc.dram_tensor("dec", [B_LOC, H], F32, kind="ExternalInput")
    ww_d = nc.dram_tensor("ww", [H, U], F32, kind="ExternalInput")
    wb_d = nc.dram_tensor("wb", [U], F32, kind="ExternalInput")
    w1_d = nc.dram_tensor("w1", [H, U], F32, kind="ExternalInput")
    w1b_d = nc.dram_tensor("w1b", [U], F32, kind="ExternalInput")
    vw_d = nc.dram_tensor("vw", [U, 1], F32, kind="ExternalInput")
    vb_d = nc.dram_tensor("vb", [1], F32, kind="ExternalInput")
    cv_d = nc.dram_tensor("cvec", [B_LOC, H], F32, kind="ExternalOutput")
    un_d = nc.dram_tensor("unnorm", [B_LOC, S], F32, kind="ExternalOutput")
    at_d = nc.dram_tensor("attn", [B_LOC, S], F32, kind="ExternalOutput")

    es = ExitStack()
    with tile.TileContext(nc) as tc:
        wp = es.enter_context(tc.tile_pool(name="wp", bufs=1))
        npool = es.enter_context(tc.tile_pool(name="npool", bufs=B_LOC))
        tpool = es.enter_context(tc.tile_pool(name="tpool", bufs=2))
        thp = es.enter_context(tc.tile_pool(name="thp", bufs=3))
        sp = es.enter_context(tc.tile_pool(name="sp", bufs=2))
        ptr = es.enter_context(tc.tile_pool(name="ptr", bufs=1, space="PSUM"))
        pmm = es.enter_context(tc.tile_pool(name="pmm", bufs=4, space="PSUM"))
        pvw = es.enter_context(tc.tile_pool(name="pvw", bufs=1, space="PSUM"))
        pcv = es.enter_context(tc.tile_pool(name="pcv", bufs=1, space="PSUM"))
        psm = es.enter_context(tc.tile_pool(name="psm", bufs=1, space="PSUM"))

        # ---- weights / setup ----
        ident16 = wp.tile([128, 128], F16)
        make_identity(nc, ident16[:])
        i4 = wp.tile([4, 4], F32)
        make_identity(nc, i4[:])

        ww32 = wp.tile([128, NK, U], F32)
        nc.sync.dma_start(out=ww32[:], in_=ww_d.ap().rearrange("(kt p) u -> p kt u", p=128))
        ww16 = wp.tile([128, NK, U], F16)
        nc.vector.tensor_copy(ww16[:], ww32[:])
        w132 = wp.tile([128, NK, U], F32)
        nc.sync.dma_start(out=w132[:], in_=w1_d.ap().rearrange("(kt p) u -> p kt u", p=128))
        vw32 = wp.tile([128, NU], F32)
        nc.sync.dma_start(out=vw32[:], in_=vw_d.ap().rearrange("(uc p) 1 -> p uc", p=128))
        vw16 = wp.tile([128, NU], F16)
        nc.vector.tensor_copy(vw16[:], vw32[:])
        wbsum = wp.tile([128, NU], F32)
        wb2 = wp.tile([128, NU], F32)
        nc.sync.dma_start(out=wbsum[:], in_=wb_d.ap().rearrange("(uc p) -> p uc", p=128))
        nc.sync.dma_start(out=wb2[:], in_=w1b_d.ap().rearrange("(uc p) -> p uc", p=128))
        nc.vector.tensor_add(wbsum[:], wbsum[:], wb2[:])
        vb_t = wp.tile([1, 1], F32)
        nc.sync.dma_start(out=vb_t[:], in_=vb_d[:])
        ones_col = wp.tile([128, 1], F32)
        nc.vector.memset(ones_col[:], 1.0)
        ones_row = wp.tile([1, 128], F32)
        nc.vector.memset(ones_row[:], 1.0)

        # dec_projT -> bias_col[u%128, uc, b] = (dec @ W1w)[b, u] + Wb[u] + W1b[u]
        dec_nat = wp.tile([B_LOC, H], F32)
        nc.sync.dma_start(out=dec_nat[:], in_=dec_d[:])
        decT = wp.tile([128, NK, B_LOC], F32)
        for kt in range(NK):
            pt0 = psm.tile([128, B_LOC], F32, tag="psm", name=f"ptd{kt}")
            nc.tensor.matmul(pt0[:], dec_nat[:, kt * 128:(kt + 1) * 128], i4[:],
                             start=True, stop=True)
            nc.scalar.activation(out=decT[:, kt, :], in_=pt0[:], func=AF.Copy)
        bias_col = wp.tile([128, NU, B_LOC], F32)
        for uc in range(NU):
            pb = psm.tile([128, B_LOC], F32, tag="psm", name=f"pbias{uc}")
            for kt in range(NK):
                nc.tensor.matmul(pb[:], w132[:, kt, uc * 128:(uc + 1) * 128],
                                 decT[:, kt, :], start=(kt == 0), stop=(kt == NK - 1))
            nc.scalar.activation(out=bias_col[:, uc, :], in_=pb[:],
                                 func=AF.Identity, bias=wbsum[:, uc:uc + 1], scale=1.0)

        # ---- prefetch all batches (fp32->fp16 cast during DMA on gpsimd) ----
        nats = []
        for b in range(B_LOC):
            nat16 = npool.tile([128, NS, H], F16, tag="nat", name=f"nat{b}")
            nc.gpsimd.dma_start(out=nat16[:],
                                in_=enc_d[b].rearrange("(si p) h -> p si h", p=128))
            nats.append(nat16)

        # ---- per-batch pipeline; c_vec of batch b-1 is emitted during batch b
        pending_cvec = [None]

        def emit_cvec():
            bb, at16 = pending_cvec[0]
            pc = pcv.tile([1, H], F32, tag="pcv", name=f"pcv{bb}")
            for si in range(NS):
                nc.tensor.matmul(pc[:], at16[:, si:si + 1], nats[bb][:, si, :],
                                 start=(si == 0), stop=(si == NS - 1))
            cv_row = sp.tile([1, H], F32, tag="cvrow", name=f"cv{bb}")
            nc.scalar.activation(out=cv_row[:], in_=pc[:], func=AF.Copy)
            nc.sync.dma_start(out=cv_d[bb:bb + 1, :], in_=cv_row[:])
            pending_cvec[0] = None

        for b in range(B_LOC):
            nat16 = nats[b]
            # transpose: encT[h%128, hc, s] via PE transpose-mode (identity stationary)
            encT = tpool.tile([128, NK, S], F16, tag="encT", name=f"encT{b}")
            for sj in range(NS // 2):        # two s-tiles per psum bank
                pt = ptr.tile([128, 2, NK, 128], F16, tag="ptr", name=f"ptr{b}_{sj}")
                for half in range(2):
                    si = sj * 2 + half
                    for hc in range(NK):
                        nc.tensor.transpose(pt[:, half, hc, :],
                                            nat16[:, si, hc * 128:(hc + 1) * 128],
                                            ident16[:])
                dst = encT[:, :, sj * 256:(sj + 1) * 256].rearrange(
                    "p hc (half q) -> p half hc q", half=2)
                if sj % 2 == 0:
                    nc.vector.tensor_copy(dst, pt[:])
                else:
                    nc.scalar.activation(out=dst, in_=pt[:], func=AF.Copy)

            un_row = sp.tile([1, S], F32, tag="unrow", name=f"un{b}")
            for sc in range(NSC):
                th = thp.tile([128, NU, SC], F16, tag="tanh", name=f"th{b}_{sc}")
                for uc in range(NU):
                    pm = pmm.tile([128, SC], F32, tag="pmm", name=f"pm{b}_{sc}_{uc}")
                    for kt in range(NK):
                        nc.tensor.matmul(pm[:], ww16[:, kt, uc * 128:(uc + 1) * 128],
                                         encT[:, kt, sc * SC:(sc + 1) * SC],
                                         start=(kt == 0), stop=(kt == NK - 1))
                    nc.scalar.activation(out=th[:, uc, :], in_=pm[:], func=AF.Tanh,
                                         bias=bias_col[:, uc, b:b + 1], scale=1.0)
                pv = pvw.tile([1, SC], F32, tag="pvw", name=f"pv{b}_{sc}")
                for uc in range(NU):
                    nc.tensor.matmul(pv[:], vw16[:, uc:uc + 1], th[:, uc, :],
                                     start=(uc == 0), stop=(uc == NU - 1))
                nc.scalar.activation(out=un_row[:, sc * SC:(sc + 1) * SC], in_=pv[:],
                                     func=AF.Identity, bias=vb_t[:], scale=1.0)
            nc.sync.dma_start(out=un_d[b:b + 1, :], in_=un_row[:])

            # previous batch's c_vec: fills the PE while softmax(b) runs
            if pending_cvec[0] is not None:
                emit_cvec()

            # softmax over all 2048 (no max-sub: |unnorm| <= sum|vw| bounded)
            u_col = sp.tile([128, NS], F32, tag="ucol", name=f"uc{b}")
            nc.sync.dma_start(out=u_col[:],
                              in_=un_d[b:b + 1, :].rearrange("1 (si p) -> p si", p=128))
            e_col = sp.tile([128, NS], F32, tag="ecol", name=f"ec{b}")
            epart = sp.tile([128, 1], F32, tag="epart", name=f"ep{b}")
            nc.scalar.activation(out=e_col[:], in_=u_col[:], func=AF.Exp,
                                 accum_out=epart[:])
            pz = psm.tile([1, 1], F32, tag="psm", name=f"pz{b}")
            nc.tensor.matmul(pz[:], ones_col[:], epart[:], start=True, stop=True)
            z_sb = sp.tile([1, 1], F32, tag="zsb", name=f"z{b}")
            nc.vector.tensor_copy(z_sb[:], pz[:])
            rz = sp.tile([1, 1], F32, tag="rz", name=f"rz{b}")
            nc.vector.reciprocal(rz[:], z_sb[:])
            prz = psm.tile([128, 1], F32, tag="psm", name=f"prz{b}")
            nc.tensor.matmul(prz[:], ones_row[:], rz[:], start=True, stop=True)
            rz_col = sp.tile([128, 1], F32, tag="rzcol", name=f"rzc{b}")
            nc.vector.tensor_copy(rz_col[:], prz[:])
            at_col = sp.tile([128, NS], F32, tag="atcol", name=f"ac{b}")
            nc.vector.tensor_scalar_mul(at_col[:], e_col[:], rz_col[:])
            at16 = sp.tile([128, NS], F16, tag="at16", name=f"a16{b}")
            nc.vector.tensor_copy(at16[:], at_col[:])
            nc.sync.dma_start(out=at_d[b:b + 1, :].rearrange("1 (si p) -> p si", p=128),
                              in_=at_col[:])
            pending_cvec[0] = (b, at16)

        emit_cvec()
        es.close()

    nc.compile()
    return nc


def get_nc():
    if "nc" not in _CACHE:
        _CACHE["nc"] = _build()
    return _CACHE["nc"]


def make_in_maps(enc_hidden, dec_hidden, Ww, Wb, W1w, W1b, vw, vb):
    enc = np.ascontiguousarray(np.asarray(enc_hidden, dtype=np.float32))
    dec = np.ascontiguousarray(np.asarray(dec_hidden, dtype=np.float32))
    ww = np.ascontiguousarray(np.asarray(Ww, dtype=np.float32))
    wb = np.ascontiguousarray(np.asarray(Wb, dtype=np.float32))
    w1 = np.ascontiguousarray(np.asarray(W1w, dtype=np.float32))
    w1b = np.ascontiguousarray(np.asarray(W1b, dtype=np.float32))
    vww = np.ascontiguousarray(np.asarray(vw, dtype=np.float32))
    vbb = np.ascontiguousarray(np.asarray(vb, dtype=np.float32))
    in_maps = []
    for c in range(NCORES):
        sl = slice(c * B_LOC, (c + 1) * B_LOC)
        in_maps.append({
            "enc": np.ascontiguousarray(enc[sl]),
            "dec": np.ascontiguousarray(dec[sl]),
            "ww": ww, "wb": wb, "w1": w1, "w1b": w1b,
            "vw": vww, "vb": vbb,
        })
    return in_maps


def kernel(enc_hidden, dec_hidden, Ww, Wb, W1w, W1b, vw, vb, _trace=False):
    from concourse.bass_utils import run_bass_kernel_spmd

    nc = get_nc()
    in_maps = make_in_maps(enc_hidden, dec_hidden, Ww, Wb, W1w, W1b, vw, vb)
    res = run_bass_kernel_spmd(nc, in_maps, list(range(NCORES)), trace=_trace)
    c_vec = np.concatenate([res.results[c]["cvec"] for c in range(NCORES)], axis=0)
    unnorm = np.concatenate([res.results[c]["unnorm"] for c in range(NCORES)], axis=0)
    attn = np.concatenate([res.results[c]["attn"] for c in range(NCORES)], axis=0)
    out = (c_vec.astype(np.float32), unnorm.astype(np.float32), attn.astype(np.float32))
    if _trace:
        return out, res
    return out
